# revision 19
# baseline (speedup 1.0000x reference)
"""GATv2 (2-layer) edge-phase kernel for 8 TRN2 NeuronCores.

Sharding: each core owns 12544 destination nodes (round-robin by degree).
Edges are bucketed by (core, 128-node window, src%4 class).

Device edge phase per window:
  - xl[src] gathered via dma_gather on 4 SWDGE queues (4 class tables of
    25088 rows each, int16 indices), rows packed [CHr | pad].
  - xr[dst] broadcast to edge slots on the tensor engine:
    psZ = UT_t^T @ xr_win, with UT (one-hot [node, slot]) shipped from host.
  - s = lrelu(xl + psZ) (vector add + scalar activation), u = s*att,
    logit = reduce_c(u), alpha = exp(logit) (scalar), msg = xl * alpha.
  - scatter: psO[n, :] += U_t^T @ cat_t via accumulating matmuls, with
    U built on-device by is_equal(dst, iota).
Host does dense linear layers, ELU, normalization, head-mean, log_softmax.
"""
import sys, os
sys.path.insert(0, "/opt/trn_rl_repo")
import numpy as np
import ml_dtypes

import concourse.bass as bass
import concourse.bacc as bacc
import concourse.mybir as mybir
import concourse.tile as tile
from concourse.bass_utils import run_bass_kernel_spmd
from concourse.library_config import mlp as mlp_lib

# ---------------- problem constants ----------------
N = 100000
E = 1600000
F_IN = 256
HID, H1, H2, NCLS = 8, 8, 4, 40
D1 = H1 * HID            # 64
D2 = H2 * NCLS           # 160
NCORES = 8
W = 98                   # windows per core
NC_N = W * 128           # 12544 nodes per core
NPAD = NCORES * NC_N     # 100352
NTAB4 = NPAD // 4        # 25088 rows per src%4 class table

BF16 = ml_dtypes.bfloat16

_cache = {}
PROFILE = []   # (exec_time_ns, trace_path) per launch, when BASS_TRACE=1


def _build_edge_program(G, TW, H, C):
    """One GAT edge phase. TW = gather row width (bf16, 256B multiple);
    real cols = CHr = H*C packed at col 0. OUTW = H + H*C."""
    T = 4 * G                # tiles of 128 edge slots per window
    CHr = H * C
    OUTW = H + CHr
    NI = G * 128             # gather indices per class
    NIC = NI // 16           # int16 idx cols per class
    nc = bacc.Bacc("TRN2", num_swdge_queues=4)
    f32, bf16, i16 = mybir.dt.float32, mybir.dt.bfloat16, mybir.dt.int16

    tabs = [nc.declare_dram_parameter(f"tab{r}", [NTAB4, TW], bf16,
                                      isOutput=False) for r in range(4)]
    xrt = nc.declare_dram_parameter("xrt", [W, 128, CHr], bf16, isOutput=False)
    gix = nc.declare_dram_parameter("gix", [W, 128, 4 * NIC], i16,
                                    isOutput=False)
    utb = nc.declare_dram_parameter("utb", [W, 128, T * 128], bf16,
                                    isOutput=False)
    dstw = nc.declare_dram_parameter("dstw", [W, 128, T], bf16, isOutput=False)
    iot = nc.declare_dram_parameter("iot", [128, 128], bf16, isOutput=False)
    idn = nc.declare_dram_parameter("idn", [128, 128], bf16, isOutput=False)
    atr = nc.declare_dram_parameter("atr", [128, CHr], bf16, isOutput=False)
    out = nc.declare_dram_parameter("out", [W, 128, OUTW], f32, isOutput=True)

    AP = bass.AP
    # psZ bank packing: CHr-wide f32 tiles per 512-f32 psum bank, 2 banks
    # per psZ pool tile (uniform stride via 4-dim AP on the bank axis).
    ZPB = 512 // CHr                 # tiles per bank: L1 8, L2 3
    ZPG = 2 * ZPB                    # tiles per psZ group tile (2 banks)
    n_grp = (T + ZPG - 1) // ZPG     # L1 2, L2 4

    with tile.TileContext(nc) as tc:
        nc.gpsimd.load_library(mlp_lib)
        with (
            tc.tile_pool(name="const", bufs=1) as pc,
            tc.tile_pool(name="idx", bufs=3) as pi,
            tc.tile_pool(name="gath", bufs=3) as pg,
            tc.tile_pool(name="ut", bufs=3) as pu,
            tc.tile_pool(name="work", bufs=3) as pw,
            tc.tile_pool(name="psz", bufs=3, space="PSUM") as pz,
            tc.tile_pool(name="pso", bufs=2, space="PSUM") as po,
        ):
            iota_sb = pc.tile([128, 128], bf16, tag="iota")
            iden_sb = pc.tile([128, 128], bf16, tag="iden")
            att_sb = pc.tile([128, CHr], bf16, tag="att")
            nc.sync.dma_start(out=iota_sb[:], in_=iot[:])
            nc.sync.dma_start(out=iden_sb[:], in_=idn[:])
            nc.sync.dma_start(out=att_sb[:], in_=atr[:])

            def s1a(w):
                """DMA loads + xl gathers (sync + gpsimd only)."""
                gidx = pi.tile([128, 4 * NIC], i16, tag="gi")
                dst_sb = pi.tile([128, T], bf16, tag="dw")
                xr_sb = pi.tile([128, CHr], bf16, tag="xr")
                ut_sb = pu.tile([128, T * 128], bf16, tag="ut")
                nc.sync.dma_start(out=gidx[:], in_=gix[w])
                nc.sync.dma_start(out=dst_sb[:], in_=dstw[w])
                nc.sync.dma_start(out=xr_sb[:], in_=xrt[w])
                nc.sync.dma_start(out=ut_sb[:], in_=utb[w])
                xlg = pg.tile([128, T * TW], bf16, tag="xlg")
                xb = xlg[:]
                gb = gidx[:]
                for r in range(4):
                    og = AP(xb.tensor, xb.offset + r * G * TW,
                            [xb.ap[0], (TW, G), (1, TW)])
                    ig = AP(gb.tensor, gb.offset + r * NIC,
                            [gb.ap[0], (1, NIC)])
                    nc.gpsimd.dma_gather(
                        out_ap=og, in_ap=tabs[r][:], idxs_ap=ig,
                        num_idxs=NI, num_idxs_reg=NI, elem_size=TW,
                        queue_num=r)
                return dict(dst=dst_sb, xr=xr_sb, ut=ut_sb, xlg=xlg)

            def s1b(st):
                """one-hot (DVE), xr-broadcast (PE), adds (DVE), lrelu."""
                U_all = pw.tile([128, 128 * T], bf16, tag="U")
                db = st["dst"][:]
                d_in = AP(db.tensor, db.offset, [db.ap[0], (1, T), (0, 128)])
                ib = iota_sb[:]
                i_in = AP(ib.tensor, ib.offset, [ib.ap[0], (0, T), (1, 128)])
                ub = U_all[:]
                u_out = AP(ub.tensor, ub.offset, [ub.ap[0], (128, T), (1, 128)])
                nc.vector.tensor_tensor(out=u_out, in0=d_in, in1=i_in,
                                        op=mybir.AluOpType.is_equal)

                # z = xl + bcast(xr) built fully in PSUM: per bank, one
                # identity matmul writes xl (start), then per-tile one-hot
                # matmuls accumulate UT_t^T @ xr_win; lrelu reads PSUM.
                s_all = pw.tile([128, T * CHr], bf16, tag="s")
                utv, xrv = st["ut"][:], st["xr"][:]
                sb_, xgb = s_all[:], st["xlg"][:]
                idv = iden_sb[:]
                for g in range(n_grp):
                    t0 = g * ZPG
                    nt = min(ZPG, T - t0)
                    ps = pz.tile([128, 1024], f32, tag="psz")
                    pb = ps[:]
                    for b in range((nt + ZPB - 1) // ZPB):
                        j0 = b * ZPB
                        nj = min(ZPB, nt - j0)
                        xl_rhs = AP(xgb.tensor, xgb.offset + (t0 + j0) * TW,
                                    [xgb.ap[0], (TW, nj), (1, CHr)])
                        zb = AP(pb.tensor, pb.offset + b * 512,
                                [pb.ap[0], (1, nj * CHr)])
                        nc.tensor.matmul(out=zb, lhsT=idv, rhs=xl_rhs,
                                         start=True, stop=False)
                        for j in range(nj):
                            t = t0 + j0 + j
                            lhsT = AP(utv.tensor, utv.offset + t * 128,
                                      [utv.ap[0], (1, 128)])
                            zout = AP(pb.tensor, pb.offset + b * 512 + j * CHr,
                                      [pb.ap[0], (1, CHr)])
                            nc.tensor.matmul(out=zout, lhsT=lhsT, rhs=xrv,
                                             start=False, stop=(j == nj - 1))
                    # lrelu PSUM -> SBUF (scalar engine)
                    segs = []
                    nfull = (nt // ZPB) * ZPB
                    if nfull:
                        segs.append((0, nt // ZPB, ZPB))
                    if nt > nfull:
                        segs.append((nfull, 1, nt - nfull))
                    for (j0, nb, nj) in segs:
                        z_in = AP(pb.tensor, pb.offset + (j0 // ZPB) * 512,
                                  [pb.ap[0], (512, nb), (CHr, nj), (1, CHr)])
                        s_out = AP(sb_.tensor, sb_.offset + (t0 + j0) * CHr,
                                   [sb_.ap[0], (CHr * ZPB, nb), (CHr, nj), (1, CHr)])
                        nc.scalar.activation(
                            out=s_out, in_=z_in,
                            func=mybir.ActivationFunctionType.Lrelu, alpha=0.2)
                st["U"] = U_all
                st["s"] = s_all

            def s2_head(st):
                """att-dot + exp (DVE + scalar)."""
                sb_ = st["s"][:]
                u_all = pw.tile([128, T * CHr], bf16, tag="u")
                logit = pw.tile([128, T * H], f32, tag="lg")
                cat = pw.tile([128, T * OUTW], bf16, tag="cat")
                ubv, lgv, cb = u_all[:], logit[:], cat[:]
                ai = AP(att_sb[:].tensor, att_sb[:].offset,
                        [att_sb[:].ap[0], (0, T), (1, CHr)])
                nc.vector.tensor_tensor(out=ubv, in0=sb_, in1=ai,
                                        op=mybir.AluOpType.mult)
                u_in = AP(ubv.tensor, ubv.offset,
                          [ubv.ap[0], (CHr, T), (C, H), (1, C)])
                nc.vector.tensor_reduce(out=lgv, in_=u_in,
                                        axis=mybir.AxisListType.X,
                                        op=mybir.AluOpType.add)
                ex_out = AP(cb.tensor, cb.offset, [cb.ap[0], (OUTW, T), (1, H)])
                nc.scalar.activation(out=ex_out, in_=lgv,
                                     func=mybir.ActivationFunctionType.Exp)
                st["cat"] = cat

            def s2_tail(w, st):
                """msg (DVE), scatter (PE), store."""
                cb = st["cat"][:]
                xgb = st["xlg"][:]
                ub = st["U"][:]
                ex_in = AP(cb.tensor, cb.offset,
                           [cb.ap[0], (OUTW, T), (1, H), (0, C)])
                m_in = AP(xgb.tensor, xgb.offset,
                          [xgb.ap[0], (TW, T), (C, H), (1, C)])
                m_out = AP(cb.tensor, cb.offset + H,
                           [cb.ap[0], (OUTW, T), (C, H), (1, C)])
                nc.vector.tensor_tensor(out=m_out, in0=m_in, in1=ex_in,
                                        op=mybir.AluOpType.mult)
                pso = po.tile([128, OUTW], f32, tag="pso")
                for t in range(T):
                    lhsT = AP(ub.tensor, ub.offset + t * 128,
                              [ub.ap[0], (1, 128)])
                    rhs = AP(cb.tensor, cb.offset + t * OUTW,
                             [cb.ap[0], (1, OUTW)])
                    nc.tensor.matmul(out=pso[:], lhsT=lhsT, rhs=rhs,
                                     start=(t == 0), stop=(t == T - 1))
                ob = pw.tile([128, OUTW], f32, tag="ob")
                nc.vector.tensor_copy(out=ob[:], in_=pso[:])
                nc.sync.dma_start(out=out[w], in_=ob[:])

            # software pipeline, gathers issued 2 windows ahead; the next
            # window's one-hot/adds fill the DVE while exp(w) round-trips
            hs = {0: s1a(0)}
            if W > 1:
                hs[1] = s1a(1)
            s1b(hs[0])
            for w in range(W):
                if w + 2 < W:
                    hs[w + 2] = s1a(w + 2)
                s2_head(hs[w])
                if w + 1 < W:
                    s1b(hs[w + 1])
                s2_tail(w, hs.pop(w))
    nc.compile()
    return nc


def _wrap16(flat):
    """[n] int -> [128, n//16] int16 (wrapped in 16 partitions, 8x repl)."""
    a = flat.reshape(-1, 16).T.astype(np.int16)
    return np.tile(a, (8, 1))


def _prep_graph(src, dst):
    """Window assignment + per-(core,window,class) edge slotting."""
    deg = np.bincount(dst, minlength=NPAD)
    order = np.argsort(-deg, kind="stable")
    wslot = np.arange(NPAD) % (NCORES * W)
    pos = np.arange(NPAD) // (NCORES * W)
    core_of = np.empty(NPAD, np.int64); w_of = np.empty(NPAD, np.int64)
    pos_of = np.empty(NPAD, np.int64)
    core_of[order] = wslot % NCORES
    w_of[order] = wslot // NCORES
    pos_of[order] = pos
    node_of = np.empty((NCORES, W, 128), np.int64)
    node_of[core_of[order], w_of[order], pos_of[order]] = order

    c_e = core_of[dst]; w_e = w_of[dst]; r_e = src % 4
    key = ((c_e * W + w_e) * 4 + r_e)
    sidx = np.argsort(key, kind="stable")
    cnt = np.bincount(key, minlength=NCORES * W * 4).reshape(NCORES, W, 4)
    G = max(5, int(np.ceil(cnt.max() / 128)))
    T = 4 * G
    NI = G * 128
    src_s, dst_s = src[sidx], dst[sidx]

    # per-(c,w,r): slot i -> partition i%128, class-tile i//128
    gidx = np.zeros((NCORES, W, 4, NI), np.int32)      # table row = src//4
    dstw = np.full((NCORES, W, 128, T), -1.0, BF16)    # pos or -1
    ut = np.zeros((NCORES, W, 128, T * 128), BF16)     # UT[n, t*128+slot]
    off = 0
    for c in range(NCORES):
        for w in range(W):
            for r in range(4):
                n = cnt[c, w, r]
                sl = slice(off, off + n); off += n
                i = np.arange(n)
                gidx[c, w, r, :n] = (src_s[sl] // 4).astype(np.int32)
                p = pos_of[dst_s[sl]]
                tt = r * G + i // 128
                ss = i % 128
                dstw[c, w, ss, tt] = p.astype(np.float32)
                ut[c, w, p, tt * 128 + ss] = 1.0
    # wrap gather indices to int16 layout [W, 128, 4*NI/16]
    gi16 = np.zeros((NCORES, W, 128, 4 * (NI // 16)), np.int16)
    for c in range(NCORES):
        for w in range(W):
            for r in range(4):
                gi16[c, w, :, r * (NI // 16):(r + 1) * (NI // 16)] = \
                    _wrap16(gidx[c, w, r])
    return dict(G=G, T=T, node_of=node_of, gi16=gi16, dstw=dstw, ut=ut,
                core_of=core_of, w_of=w_of, pos_of=pos_of)


def _run_layer(gp, xl_full, xr_full, att, H, C):
    """xl_full [NPAD, CHr] f32 (global node order), xr_full same. Returns
    den [NPAD, H], msg [NPAD, H, C] f32."""
    G, T = gp["G"], gp["T"]
    CHr = H * C
    TW = 128 * ((CHr * 2 + 255) // 256)   # row bytes multiple of 256
    OUTW = H + CHr

    # class tables: row n//4 of class n%4, packed CHr cols
    tabw = np.zeros((4, NTAB4, TW), BF16)
    xl_b = xl_full.astype(BF16)
    for r in range(4):
        tabw[r, :, :CHr] = xl_b[r::4]
    node_of = gp["node_of"]
    att_c = np.tile(att.reshape(1, CHr), (128, 1)).astype(BF16)
    iota = np.tile(np.arange(128, dtype=np.float32), (128, 1)).astype(BF16)
    iden = np.eye(128, dtype=np.float32).astype(BF16)

    in_maps = []
    for c in range(NCORES):
        xr_rows = xr_full[node_of[c].reshape(-1)].astype(BF16)
        in_maps.append(dict(
            tab0=tabw[0], tab1=tabw[1], tab2=tabw[2], tab3=tabw[3],
            xrt=np.ascontiguousarray(xr_rows.reshape(W, 128, CHr)),
            gix=np.ascontiguousarray(gp["gi16"][c]),
            utb=np.ascontiguousarray(gp["ut"][c]),
            dstw=np.ascontiguousarray(gp["dstw"][c]),
            iot=iota, idn=iden, atr=att_c,
        ))

    key = (G, TW, H, C)
    if key not in _cache:
        _cache[key] = _build_edge_program(G, TW, H, C)
    nc = _cache[key]
    res = run_bass_kernel_spmd(nc, in_maps, list(range(NCORES)))
    PROFILE.append((res.exec_time_ns,
                    res.instructions_and_trace[1] if res.instructions_and_trace else None))
    den = np.zeros((NPAD, H), np.float32)
    msg = np.zeros((NPAD, H, C), np.float32)
    for c in range(NCORES):
        o = res.results[c]["out"].reshape(NC_N, OUTW)
        nodes = node_of[c].reshape(-1)
        den[nodes] = o[:, :H]
        msg[nodes] = o[:, H:].reshape(NC_N, H, C)
    return den, msg


def kernel(x, edge_index, Wl1, bl1, Wr1, br1, att1, b1,
           Wl2, bl2, Wr2, br2, att2, b2):
    x = np.asarray(x, np.float32)
    ei = np.asarray(edge_index).astype(np.int64)
    loop = np.arange(N, dtype=np.int64)
    src = np.concatenate([ei[0], loop])
    dst = np.concatenate([ei[1], loop])
    gp = _prep_graph(src, dst)

    xl1 = np.zeros((NPAD, D1), np.float32)
    xr1 = np.zeros((NPAD, D1), np.float32)
    xl1[:N] = x @ np.asarray(Wl1, np.float32) + np.asarray(bl1, np.float32)
    xr1[:N] = x @ np.asarray(Wr1, np.float32) + np.asarray(br1, np.float32)
    den1, msg1 = _run_layer(gp, xl1, xr1, np.asarray(att1, np.float32), H1, HID)
    out1 = msg1.reshape(NPAD, D1)[:N] / np.maximum(den1[:N].repeat(HID, 1), 1e-16)
    h = out1 + np.asarray(b1, np.float32)
    h = np.where(h > 0, h, np.expm1(h))          # ELU

    xl2 = np.zeros((NPAD, D2), np.float32)
    xr2 = np.zeros((NPAD, D2), np.float32)
    xl2[:N] = h @ np.asarray(Wl2, np.float32) + np.asarray(bl2, np.float32)
    xr2[:N] = h @ np.asarray(Wr2, np.float32) + np.asarray(br2, np.float32)
    den2, msg2 = _run_layer(gp, xl2, xr2, np.asarray(att2, np.float32), H2, NCLS)
    out2 = msg2[:N] / np.maximum(den2[:N, :, None], 1e-16)   # [N, H2, NCLS]
    o = out2.mean(1) + np.asarray(b2, np.float32)
    o = o - o.max(1, keepdims=True)
    o = o - np.log(np.exp(o).sum(1, keepdims=True))
    return o.astype(np.float32)
